# revision 70
# baseline (speedup 1.0000x reference)
"""Trainium2 Bass kernel for a dense pre-norm transformer block.

B, S, H, NH, MLP = 4, 2048, 768, 12, 3072 (fp32 I/O).

Sharding: 8 shards = (batch, seq-half). Each core receives its batch's full
2048-token sequence with its own 1024 query tokens permuted to the front
(attention is permutation-invariant over keys), computes K/V for all 2048
tokens, and Q/attention/MLP for its 1024 query tokens. No collectives.

On-chip: activations are kept feature-major [feature-part, token-free] for
matmuls (weights stationary), token-major for LN/softmax-normalize/residual.
Attention computes scoresT = K @ Q^T per head, exponentiates on ACT
(scale=1/8 folded), then multiplies with a stationary [V | 1/64] so the
softmax denominator accumulates for free in the extra PSUM row; the
normalization happens after a PE transpose back to token-major where the
denominator is a per-partition scalar.

fp8 (e4m3) with DoubleRow perf mode (2 k-tiles per matmul) is used for the
large-contraction GEMMs: Q/K/V/O projections, the probs@V ctx matmul, and
the MLP second linear. Weights are pre-scaled x256 host-side (w std 0.02 is
subnormal in e4m3); the 1/256 dequant rides the PSUM-evacuation ops. Scores
(64-deep contraction) and the MLP first linear (error-critical: feeds gelu)
stay bf16. fp32 accumulation everywhere, fp32 LN/residual spine.

Schedule: the attention kv-loop is software-pipelined (scores run one
kv-pair ahead of the ctx matmul consuming their exp) and the PE slack under
the ACT-bound exp stream is filled with "filler" thunks: the next head
pair's QK projection, the previous head's softmax-normalize, and the
ctx->feature-major transposes feeding the out-projection. The attention
window runs at the ACT exp floor (~100% ACT busy); the MLP phase at ~99% PE.
"""

import sys

if "/opt/trn_rl_repo" not in sys.path:
    sys.path.insert(0, "/opt/trn_rl_repo")

from contextlib import ExitStack

import ml_dtypes
import numpy as np

import concourse.bacc as bacc
import concourse.bass as bass
import concourse.mybir as mybir
import concourse.tile as tile
from concourse.alu_op_type import AluOpType
from concourse.bass_utils import run_bass_kernel_spmd
from concourse.masks import make_identity

B, S, H, NH, MLPD = 4, 2048, 768, 12, 3072
HD = H // NH  # 64
EPS = 1e-6
P = 128
N_H = H // P  # 6
N_M = MLPD // P  # 24
AF = mybir.ActivationFunctionType
BF = mybir.dt.bfloat16
F32 = mybir.dt.float32
F8 = mybir.dt.float8e4
DR = mybir.MatmulPerfMode.DoubleRow
NPF8 = ml_dtypes.float8_e4m3
WS = 256.0  # weight pre-scale (host)
CS = 64.0  # ctx scale via 1/64 ones-column

_BUILD_CACHE = {}


def build(tkv=S, mlp_act="Gelu", ln_trivial=True, bias_trivial=True):
    key = (tkv, mlp_act, ln_trivial, bias_trivial)
    if key in _BUILD_CACHE:
        return _BUILD_CACHE[key]

    tq = tkv // 2
    n_kv = tkv // P  # K/V token tiles
    n_q = tq // P  # query token tiles
    CH = 512 if tq % 512 == 0 else tq  # moving-operand chunk
    n_cq = tq // CH  # query chunks
    n_ckv = tkv // CH  # kv chunks
    n_b = CH // P  # 128-blocks per chunk
    VC = 384  # v-proj output chunk (6 heads)
    n_vc = H // VC  # 2

    nc = bacc.Bacc("TRN2", target_bir_lowering=False, debug=False, num_devices=8)

    x_d = nc.dram_tensor("x_loc", (tkv, H), BF, kind="ExternalInput").ap()
    wq_d = nc.dram_tensor("wq", (H, H), F8, kind="ExternalInput").ap()
    wk_d = nc.dram_tensor("wk", (H, H), F8, kind="ExternalInput").ap()
    wv_d = nc.dram_tensor("wv", (H, H), F8, kind="ExternalInput").ap()
    wo_d = nc.dram_tensor("wo", (H, H), F8, kind="ExternalInput").ap()
    # MLP first linear: leading 4 k-tiles fp8 (x256), trailing 2 bf16
    # (also x256 -- lossless exponent shift -- so the psum shares one scale)
    M1A = 4
    w1a_d = nc.dram_tensor("w1a", (M1A * P, MLPD), F8, kind="ExternalInput").ap()
    w1b_d = nc.dram_tensor("w1b", ((N_H - M1A) * P, MLPD), BF, kind="ExternalInput").ap()
    w2_d = nc.dram_tensor("w2", (MLPD, H), F8, kind="ExternalInput").ap()
    bq_d = nc.dram_tensor("bq", (H,), F32, kind="ExternalInput").ap()
    bk_d = nc.dram_tensor("bk", (H,), F32, kind="ExternalInput").ap()
    bv_d = nc.dram_tensor("bv", (H,), BF, kind="ExternalInput").ap()
    bo_d = nc.dram_tensor("bo", (H,), F32, kind="ExternalInput").ap()
    b1_d = nc.dram_tensor("b1", (MLPD,), F32, kind="ExternalInput").ap()
    b2_d = nc.dram_tensor("b2", (H,), F32, kind="ExternalInput").ap()
    ln1w_d = nc.dram_tensor("ln1_w", (H,), BF, kind="ExternalInput").ap()
    ln1b_d = nc.dram_tensor("ln1_b", (H,), BF, kind="ExternalInput").ap()
    ln2w_d = nc.dram_tensor("ln2_w", (H,), BF, kind="ExternalInput").ap()
    ln2b_d = nc.dram_tensor("ln2_b", (H,), BF, kind="ExternalInput").ap()
    out_d = nc.dram_tensor("out_loc", (tq, H), F32, kind="ExternalOutput").ap()

    def bcast(ap1d):
        return bass.AP(
            tensor=ap1d.tensor, offset=ap1d.offset, ap=[[0, P]] + list(ap1d.ap)
        )

    with tile.TileContext(nc) as tc, ExitStack() as top:
        const = top.enter_context(tc.tile_pool(name="const", bufs=1))
        persist = top.enter_context(tc.tile_pool(name="persist", bufs=1))
        # Top-level PSUM pool: 2 banks shared by transposes + proj accums.
        psum = top.enter_context(tc.tile_pool(name="psum", bufs=1, space="PSUM"))
        toks = top.enter_context(tc.tile_pool(name="toks", bufs=4))
        tmps = top.enter_context(tc.tile_pool(name="tmps", bufs=2))

        # ---- constants ----
        ident = const.tile([P, P], BF)
        make_identity(nc, ident)
        eps_t = const.tile([P, 1], F32)
        nc.vector.memset(eps_t, EPS)
        ln1w_bc = const.tile([P, H], BF)
        nc.gpsimd.dma_start(out=ln1w_bc, in_=bcast(ln1w_d))
        ln1b_bc = const.tile([P, H], BF)
        nc.gpsimd.dma_start(out=ln1b_bc, in_=bcast(ln1b_d))
        ln2w_bc = const.tile([P, H], BF)
        nc.gpsimd.dma_start(out=ln2w_bc, in_=bcast(ln2w_d))
        ln2b_bc = const.tile([P, H], BF)
        nc.gpsimd.dma_start(out=ln2b_bc, in_=bcast(ln2b_d))
        bv_row = const.tile([1, H], BF)
        nc.sync.dma_start(out=bv_row, in_=bv_d[None, :])
        ones_row = const.tile([1, P], BF)
        nc.vector.memset(ones_row, 1.0)
        bq_sb = const.tile([P, N_H], F32)
        nc.sync.dma_start(out=bq_sb, in_=bq_d.rearrange("(t p) -> p t", p=P))
        bk_sb = const.tile([P, N_H], F32)
        nc.sync.dma_start(out=bk_sb, in_=bk_d.rearrange("(t p) -> p t", p=P))
        bo_sb = const.tile([P, N_H], F32)
        nc.sync.dma_start(out=bo_sb, in_=bo_d.rearrange("(t p) -> p t", p=P))
        b1_sb = const.tile([P, N_M], F32)
        nc.sync.dma_start(out=b1_sb, in_=b1_d.rearrange("(t p) -> p t", p=P))
        b2_sb = const.tile([P, N_H], F32)
        nc.sync.dma_start(out=b2_sb, in_=b2_d.rearrange("(t p) -> p t", p=P))
        wo_sb = const.tile([P, N_H, H], F8)

        ctx_tok = persist.tile([P, n_q, H], BF)  # 64*normalized ctx (token-major)
        ctxT = persist.tile([P, N_H, tq], F8)  # feature-major ctx for O-proj

        def ln_tile(x_ap, w_bc, b_bc, out_bf):
            """LayerNorm of one [P, H] tile -> bf16 out (token-major).

            Stats on DVE, the normalize pass on ACT (per-partition
            scale/bias). When the LN weight/bias are the identity (this
            problem's setup), the ACT pass writes the result directly.
            """
            stats = tmps.tile([P, 2, 6], F32, tag="ln_stats", bufs=4)
            for g in range(2):
                nc.vector.bn_stats(out=stats[:, g, :], in_=x_ap[:, g * 384 : (g + 1) * 384])
            mv = tmps.tile([P, 2], F32, tag="ln_mv", bufs=4)
            nc.vector.bn_aggr(out=mv, in_=stats)
            rstd = tmps.tile([P, 1], F32, tag="ln_rstd", bufs=4)
            nc.scalar.activation(out=rstd, in_=mv[:, 1:2], func=AF.Sqrt, bias=eps_t, scale=1.0)
            nc.vector.reciprocal(out=rstd, in_=rstd)
            nmr = tmps.tile([P, 1], F32, tag="ln_nmr", bufs=4)
            nc.vector.scalar_tensor_tensor(
                out=nmr, in0=mv[:, 0:1], scalar=-1.0, in1=rstd,
                op0=AluOpType.mult, op1=AluOpType.mult,
            )
            if ln_trivial:
                nc.scalar.activation(
                    out=out_bf, in_=x_ap, func=AF.Identity, scale=rstd, bias=nmr
                )
            else:
                xh = tmps.tile([P, H], BF, tag="ln_xh", bufs=4)
                nc.scalar.activation(out=xh, in_=x_ap, func=AF.Identity, scale=rstd, bias=nmr)
                nc.vector.tensor_mul(out_bf, xh, w_bc)
                nc.vector.tensor_add(out_bf, out_bf, b_bc)

        def transpose_to(dst_ap, src_ap, rows, cols, copy_eng="scalar"):
            """dst[cols, rows] = src[rows, cols].T (both SBUF; dst may narrow)."""
            pt = psum.tile([P, P], BF, tag="aux", bufs=2)
            nc.tensor.transpose(pt[0:cols, 0:rows], src_ap, ident[0:rows, 0:rows])
            if copy_eng == "scalar":
                nc.scalar.copy(out=dst_ap, in_=pt[0:cols, 0:rows])
            else:
                nc.vector.tensor_copy(out=dst_ap, in_=pt[0:cols, 0:rows])

        # ====== Phase 1-3: LN1, V proj, then per head-pair (QK proj +
        # attention) so PE-dense projection work fills exp-wait gaps. ======
        ACH = min(1024, tq)
        n_ac = tq // ACH
        n_sc = ACH // CH
        with tc.tile_pool(name="qkv_sb", bufs=1) as qkv_sb:
            # Q stored zero-padded per head: head h occupies its 64 rows,
            # the other 64 rows stay zero, so the scores matmul can use the
            # full 128-row kT stationary (FWL) with exact math.
            # Q stored zero-padded per head: head h occupies its 64 rows,
            # the other 64 rows stay zero, so the scores matmul runs the
            # full 128-row stationary (FWL + back-to-back pipelining beat
            # the dense 64-row form by ~2x on hardware).
            qT = qkv_sb.tile([P, NH, tq], BF)
            kT = qkv_sb.tile([P, N_H, tkv], BF)
            # per-head row padded 65->68 so the kv-tile stride (12*68=816B)
            # meets DoubleRow's step%16==0 ldweights requirement
            VP = 68
            vone = qkv_sb.tile([P, n_kv, NH, VP], F8)
            # qT padding rows zeroed on the (otherwise idle) gpsimd engine;
            # finishes long before the first q evac at ~LN1-phase end
            nc.gpsimd.memset(qT, 0.0)

            with tc.tile_pool(name="ln_qkv", bufs=1) as lnp, tc.tile_pool(
                name="attn_sb", bufs=1
            ) as asb:
                xnT = lnp.tile([P, N_H, tkv], F8)
                wq_sb = lnp.tile([P, N_H, H], F8)
                wk_sb = lnp.tile([P, N_H, H], F8)
                wv_sb = lnp.tile([P, N_H, H], F8)
                for i in range(N_H):
                    nc.sync.dma_start(out=wv_sb[:, i, :], in_=wv_d[i * P : (i + 1) * P, :])

                def qk_group(w_sb, b_sb, dstT, hot, c, split_q=False):
                    pk = psum.tile([P, CH], F32, tag="aux", bufs=2)
                    for hit in range(0, N_H, 2):
                        nc.tensor.matmul(
                            pk,
                            w_sb[:, hit : hit + 2, hot * P : (hot + 1) * P],
                            xnT[:, hit : hit + 2, c * CH : (c + 1) * CH],
                            start=(hit == 0), stop=(hit == N_H - 2),
                            perf_mode=DR,
                        )
                    if split_q:
                        nc.vector.tensor_scalar(
                            dstT[0:HD, 2 * hot, c * CH : (c + 1) * CH],
                            pk[0:HD, :], 1.0 / WS,
                            b_sb[:, hot : hot + 1][0:HD],
                            AluOpType.mult, AluOpType.add,
                        )
                        nc.vector.tensor_scalar(
                            dstT[HD:P, 2 * hot + 1, c * CH : (c + 1) * CH],
                            pk[HD:P, :], 1.0 / WS,
                            b_sb[:, hot : hot + 1][HD:P],
                            AluOpType.mult, AluOpType.add,
                        )
                    else:
                        nc.vector.tensor_scalar(
                            dstT[:, hot, c * CH : (c + 1) * CH], pk,
                            1.0 / WS, b_sb[:, hot : hot + 1],
                            AluOpType.mult, AluOpType.add,
                        )

                # LN1 + transpose + V projection, per token tile (keeps PE
                # fed with V matmuls while DVE/ACT chew the next LN). The
                # attention PSUM pool opens only after this loop, so V
                # accumulators and transposes get their own banks here.
                with tc.tile_pool(name="psLN", bufs=1, space="PSUM") as psLN:
                    for t in range(n_kv):
                        x_t = toks.tile([P, H], BF, tag="xtok")
                        nc.sync.dma_start(out=x_t, in_=x_d[t * P : (t + 1) * P, :])
                        xn_bf = tmps.tile([P, H], BF, tag="xn_bf", bufs=4)
                        ln_tile(x_t, ln1w_bc, ln1b_bc, xn_bf)
                        # paired evacuation: two transposes share one strided
                        # copy, halving the per-copy fixed latency. DVE (the
                        # LN1-phase limiter) gets only one of the three.
                        for j2 in range(0, N_H, 2):
                            pt = psum.tile([P, 2, P], BF, tag="aux", bufs=2)
                            nc.tensor.transpose(
                                pt[:, 0, :], xn_bf[:, j2 * P : (j2 + 1) * P], ident
                            )
                            nc.tensor.transpose(
                                pt[:, 1, :], xn_bf[:, (j2 + 1) * P : (j2 + 2) * P], ident
                            )
                            dst = xnT[:, j2 : j2 + 2, t * P : (t + 1) * P]
                            if j2 < 4:
                                nc.scalar.copy(out=dst, in_=pt)
                            else:
                                nc.vector.tensor_copy(out=dst, in_=pt)
                        for c2 in range(n_vc):
                            pv = psLN.tile([P, VC], F32, tag="pv", bufs=3)
                            if not bias_trivial:
                                # bias row via K=1 ones-matmul (bv pre-scaled
                                # x256 host-side)
                                nc.tensor.matmul(
                                    pv, ones_row[:, 0:P],
                                    bv_row[:, c2 * VC : (c2 + 1) * VC],
                                    start=True, stop=False,
                                )
                            for hit in range(0, N_H, 2):
                                nc.tensor.matmul(
                                    pv,
                                    xnT[:, hit : hit + 2, t * P : (t + 1) * P],
                                    wv_sb[:, hit : hit + 2, c2 * VC : (c2 + 1) * VC],
                                    start=(bias_trivial and hit == 0),
                                    stop=(hit == N_H - 2),
                                    perf_mode=DR,
                                )
                            nc.vector.tensor_scalar_mul(
                                vone[:, t, c2 * (VC // HD) : (c2 + 1) * (VC // HD), 0:HD],
                                pv.rearrange("p (h d) -> p h d", d=HD),
                                1.0 / WS,
                            )

                # Q/K/O weights only needed once attention starts; emit
                # their loads after the x/LN traffic so they don't delay it.
                for i in range(N_H):
                    nc.sync.dma_start(out=wq_sb[:, i, :], in_=wq_d[i * P : (i + 1) * P, :])
                    nc.sync.dma_start(out=wk_sb[:, i, :], in_=wk_d[i * P : (i + 1) * P, :])
                    nc.sync.dma_start(out=wo_sb[:, i, :], in_=wo_d[i * P : (i + 1) * P, :])
                # the vone ones-column (tiny) on DVE
                nc.vector.memset(vone[:, :, :, HD : HD + 1], 1.0 / CS)

                def qk_groups(ht):
                    """QK projection for head-pair ht as filler thunks."""
                    for c in range(n_cq):
                        yield lambda c=c: qk_group(wq_sb, bq_sb, qT, ht, c, True)
                    for c in range(n_ckv):
                        yield lambda c=c: qk_group(wk_sb, bk_sb, kT, ht, c)

                psA = []

                def attention_head(h, fillers):
                    """Scores+exp+ctx for head h; pops one filler thunk per
                    kv-pair so the PE queue always has ready work while ACT
                    runs exp. Returns deferred normalize thunks."""
                    ht = h // 2
                    n_p = n_kv // 2
                    norm = []
                    for c in range(n_ac):
                        pctx = psA[0].tile([P, ACH], F32, tag="pctx", bufs=1)

                        def do_scores(kt2, c=c):
                            expair = asb.tile([P, 2, ACH], F8, tag="exp", bufs=4)
                            for j2 in range(2):
                                kt = 2 * kt2 + j2
                                ps = psA[0].tile([P, ACH], F32, tag="psc", bufs=2)
                                for sc in range(n_sc):
                                    nc.tensor.matmul(
                                        ps[:, sc * CH : (sc + 1) * CH],
                                        kT[:, ht, kt * P : (kt + 1) * P],
                                        qT[:, h,
                                           c * ACH + sc * CH : c * ACH + (sc + 1) * CH],
                                        start=True, stop=True,
                                    )
                                nc.scalar.activation(
                                    out=expair[:, j2, :], in_=ps, func=AF.Exp, scale=0.125
                                )
                            return expair

                        def do_ctx(kt2, expair, pctx=pctx):
                            for sc in range(n_sc):
                                nc.tensor.matmul(
                                    pctx[0 : HD + 1, sc * CH : (sc + 1) * CH],
                                    vone[:, 2 * kt2 : 2 * kt2 + 2, h, 0 : HD + 1],
                                    expair[:, :, sc * CH : (sc + 1) * CH],
                                    start=(kt2 == 0), stop=(kt2 == n_p - 1),
                                    perf_mode=DR,
                                )

                        # software pipeline: scores run one kv-pair ahead of
                        # the ctx matmul that consumes its exp; filler thunks
                        # (QK-proj / prev-head normalize) plug the PE slack.
                        prev = do_scores(0)
                        for kt2 in range(1, n_p):
                            cur = do_scores(kt2)
                            do_ctx(kt2 - 1, prev)
                            prev = cur
                            if fillers:
                                fillers.popleft()()
                        do_ctx(n_p - 1, prev)
                        cd = asb.tile([P, ACH], BF, tag="cd", bufs=3)
                        nc.vector.tensor_copy(out=cd[0 : HD + 1, :], in_=pctx[0 : HD + 1, :])

                        def norm_chunk(b4s, c=c, cd=cd):
                            for b4 in b4s:
                                t_tok = c * (ACH // P) + b4
                                pt = psum.tile([P, P], BF, tag="aux", bufs=2)
                                nc.tensor.transpose(
                                    pt[0:P, 0 : HD + 1],
                                    cd[0 : HD + 1, b4 * P : (b4 + 1) * P],
                                    ident[0 : HD + 1, 0 : HD + 1],
                                )
                                rp = tmps.tile([P, 1], F32, tag="rp", bufs=4)
                                nc.vector.reciprocal(rp, pt[:, HD : HD + 1])
                                nc.vector.tensor_scalar_mul(
                                    ctx_tok[:, t_tok, h * HD : (h + 1) * HD],
                                    pt[:, 0:HD],
                                    rp,
                                )

                        nb4 = ACH // P
                        for i0 in range(0, nb4, 2):
                            norm.append(lambda i0=i0: norm_chunk(range(i0, min(i0 + 2, nb4))))
                    return norm

                def ctxT_chunks(ht):
                    """Transpose the normalized pair-ht ctx into feature-major
                    ctxT (O-proj stationary input) as filler thunks."""
                    def one(t0):
                        for t in range(t0, min(t0 + 2, n_q)):
                            transpose_to(
                                ctxT[:, ht, t * P : (t + 1) * P],
                                ctx_tok[:, t, ht * P : (ht + 1) * P], P, P,
                                copy_eng="vector",
                            )
                    for t0 in range(0, n_q, 2):
                        yield lambda t0=t0: one(t0)

                from collections import deque

                with tc.tile_pool(name="psA", bufs=1, space="PSUM") as psA_:
                    psA.append(psA_)
                    # pair 0's QK-proj has no attention to hide in: run now.
                    for f in qk_groups(0):
                        f()
                    carried = deque()
                    for ht in range(N_H):
                        fillers = deque()
                        if ht + 1 < N_H:
                            fillers.extend(qk_groups(ht + 1))
                        fillers.extend(carried)
                        carried = deque()
                        norm0 = attention_head(2 * ht, fillers)
                        # the even head's normalize can fill the odd head's
                        # slots within the same pair
                        fillers.extend(norm0)
                        norm1 = attention_head(2 * ht + 1, fillers)
                        # leftover fillers (incl. prev pair's normalize) now
                        while fillers:
                            fillers.popleft()()
                        carried.extend(norm1)
                        carried.extend(ctxT_chunks(ht))
                    while carried:
                        carried.popleft()()

        # ========== Phase 4-6 ==========
        with tc.tile_pool(name="late", bufs=1) as late:
            x1_sb = late.tile([P, n_q, H], F32)  # attn-block out (token-major)
            # prefetch the residual x tiles so the out-proj phase never
            # waits on DMA. On the gpsimd queue: the wait for attention-phase
            # SBUF to free must not block the sync queue (w1/w2 loads).
            xr_all = late.tile([P, n_q, H], BF)
            for t in range(n_q):
                nc.gpsimd.dma_start(out=xr_all[:, t, :], in_=x_d[t * P : (t + 1) * P, :])

            # ---- out-proj + residual + LN2 (chunked), then MLP ----
            # The MLP pool opens before the out-proj loop so each chunk's
            # LN2 pass runs right after its residual: MLP chunk 0 is ready
            # while chunk 1's out-projection still runs. (The w1/w2 loads
            # block the sync queue on SBUF-free, but nothing else needs it
            # until the output DMAs.)
            with tc.tile_pool(name="mlp_sb", bufs=1) as mp, tc.tile_pool(
                name="ps6", bufs=1, space="PSUM"
            ) as ps6:
                xn2T8 = mp.tile([P, M1A, tq], F8)
                xn2T = mp.tile([P, N_H - M1A, tq], BF)
                w1a_sb = mp.tile([P, M1A, MLPD], F8)
                w1b_sb = mp.tile([P, N_H - M1A, MLPD], BF)
                w2_sb = mp.tile([P, N_M, H], F8)
                h1c = mp.tile([P, N_M, CH], F8)
                y2T = mp.tile([P, N_H, CH], BF)
                for i in range(M1A):
                    nc.sync.dma_start(out=w1a_sb[:, i, :], in_=w1a_d[i * P : (i + 1) * P, :])
                for i in range(N_H - M1A):
                    nc.sync.dma_start(out=w1b_sb[:, i, :], in_=w1b_d[i * P : (i + 1) * P, :])
                for i in range(N_M):
                    nc.sync.dma_start(out=w2_sb[:, i, :], in_=w2_d[i * P : (i + 1) * P, :])

                def ln2_tile(t):
                    xn2_bf = tmps.tile([P, H], BF, tag="xn_bf", bufs=4)
                    ln_tile(x1_sb[:, t, :], ln2w_bc, ln2b_bc, xn2_bf)
                    # paired evacuation (see xnT): pairs (0,1),(2,3) land in
                    # the fp8 tile, (4,5) in the bf16 tile
                    for j2 in range(0, N_H, 2):
                        pt = psum.tile([P, 2, P], BF, tag="aux", bufs=2)
                        nc.tensor.transpose(
                            pt[:, 0, :], xn2_bf[:, j2 * P : (j2 + 1) * P], ident
                        )
                        nc.tensor.transpose(
                            pt[:, 1, :], xn2_bf[:, (j2 + 1) * P : (j2 + 2) * P], ident
                        )
                        dst = (
                            xn2T8[:, j2 : j2 + 2, t * P : (t + 1) * P]
                            if j2 < M1A
                            else xn2T[:, 0:2, t * P : (t + 1) * P]
                        )
                        nc.vector.tensor_copy(out=dst, in_=pt)

                uT = mp.tile([P, N_H, tq], BF)

                def o_chunk(c):
                    for hot in range(N_H):
                        pu = psum.tile([P, CH], F32, tag="aux", bufs=2)
                        for hit in range(0, N_H, 2):
                            nc.tensor.matmul(
                                pu,
                                wo_sb[:, hit : hit + 2, hot * P : (hot + 1) * P],
                                ctxT[:, hit : hit + 2, c * CH : (c + 1) * CH],
                                start=(hit == 0), stop=(hit == N_H - 2),
                                perf_mode=DR,
                            )
                        nc.vector.tensor_scalar(
                            uT[:, hot, c * CH : (c + 1) * CH], pu,
                            1.0 / (WS * CS), bo_sb[:, hot : hot + 1],
                            AluOpType.mult, AluOpType.add,
                        )

                def res_ln2(c):
                    for t in range(c * n_b, (c + 1) * n_b):
                        for j2 in range(0, N_H, 2):
                            pt = psum.tile([P, 2, P], BF, tag="aux", bufs=2)
                            nc.tensor.transpose(
                                pt[:, 0, :], uT[:, j2, t * P : (t + 1) * P], ident,
                            )
                            nc.tensor.transpose(
                                pt[:, 1, :], uT[:, j2 + 1, t * P : (t + 1) * P], ident,
                            )
                            nc.vector.tensor_add(
                                x1_sb[:, t, j2 * P : (j2 + 2) * P],
                                pt,
                                xr_all[:, t, j2 * P : (j2 + 2) * P],
                            )
                    for t in range(c * n_b, (c + 1) * n_b):
                        ln2_tile(t)

                def mlp_ph(c):
                    for mt in range(N_M):
                        ph = ps6.tile([P, CH], F32, tag="pmm", bufs=4)
                        for hit in range(0, M1A, 2):
                            nc.tensor.matmul(
                                ph,
                                w1a_sb[:, hit : hit + 2, mt * P : (mt + 1) * P],
                                xn2T8[:, hit : hit + 2, c * CH : (c + 1) * CH],
                                start=(hit == 0), stop=False,
                                perf_mode=DR,
                            )
                        for hit in range(N_H - M1A):
                            nc.tensor.matmul(
                                ph,
                                w1b_sb[:, hit, mt * P : (mt + 1) * P],
                                xn2T[:, hit, c * CH : (c + 1) * CH],
                                start=False, stop=(hit == N_H - M1A - 1),
                            )
                        nc.scalar.activation(
                            out=h1c[:, mt, :], in_=ph,
                            func=getattr(AF, mlp_act), bias=b1_sb[:, mt : mt + 1],
                            scale=1.0 / WS,
                        )

                def mlp_tail(c):
                    for hot in range(N_H):
                        py = ps6.tile([P, CH], F32, tag="pmm", bufs=4)
                        for mt in range(0, N_M, 2):
                            nc.tensor.matmul(
                                py,
                                w2_sb[:, mt : mt + 2, hot * P : (hot + 1) * P],
                                h1c[:, mt : mt + 2, :],
                                start=(mt == 0), stop=(mt == N_M - 2),
                                perf_mode=DR,
                            )
                        nc.vector.tensor_scalar(
                            y2T[:, hot, :], py, 1.0 / WS, b2_sb[:, hot : hot + 1],
                            AluOpType.mult, AluOpType.add,
                        )
                    for b4 in range(n_b):
                        t = c * n_b + b4
                        outt = toks.tile([P, H], F32, tag="xtok")
                        for j2 in range(0, N_H, 2):
                            pt = psum.tile([P, 2, P], BF, tag="aux", bufs=2)
                            nc.tensor.transpose(
                                pt[:, 0, :], y2T[:, j2, b4 * P : (b4 + 1) * P], ident,
                            )
                            nc.tensor.transpose(
                                pt[:, 1, :], y2T[:, j2 + 1, b4 * P : (b4 + 1) * P], ident,
                            )
                            nc.vector.tensor_add(
                                outt[:, j2 * P : (j2 + 2) * P],
                                pt,
                                x1_sb[:, t, j2 * P : (j2 + 2) * P],
                            )
                        # alternate DMA queues so the output drain overlaps
                        eng = nc.sync if t % 2 == 0 else nc.gpsimd
                        eng.dma_start(out=out_d[t * P : (t + 1) * P, :], in_=outt)

                # interleaved emission: both out-proj chunks back-to-back
                # (36 DR matmuls keep PE dense while DVE evacuates), then
                # MLP chunk 0's first-linear groups fill the PE dip while
                # chunk 1's residual/LN2 chain resolves.
                o_chunk(0)
                o_chunk(1)
                res_ln2(0)
                res_ln2(1)  # fills the PE stall while chunk 0's LN2 resolves
                mlp_ph(0)
                mlp_tail(0)
                mlp_ph(1)
                mlp_tail(1)

    nc.compile()
    _BUILD_CACHE[key] = nc
    return nc


def make_in_maps(inputs, tkv=S):
    """Build the 8 per-core input maps from full inputs."""
    f = np.asarray
    x = f(inputs["x"], dtype=np.float32)
    tq = tkv // 2
    wcast = {
        n: np.ascontiguousarray((f(inputs[n], dtype=np.float32) * WS).astype(NPF8))
        for n in ["wq", "wk", "wv", "wo", "w2"]
    }
    w1f = f(inputs["w1"], dtype=np.float32) * WS
    wcast["w1a"] = np.ascontiguousarray(w1f[: 4 * P].astype(NPF8))
    wcast["w1b"] = np.ascontiguousarray(w1f[4 * P :].astype(ml_dtypes.bfloat16))
    fp32v = {
        n: np.ascontiguousarray(f(inputs[n], dtype=np.float32))
        for n in ["bq", "bk", "bo", "b1", "b2"]
    }
    for n in ["ln1_w", "ln1_b", "ln2_w", "ln2_b"]:
        fp32v[n] = np.ascontiguousarray(f(inputs[n]).astype(ml_dtypes.bfloat16))
    # bv rides a ones-matmul row inside the x256-scaled V psum
    fp32v["bv"] = np.ascontiguousarray(
        (f(inputs["bv"], dtype=np.float32) * WS).astype(ml_dtypes.bfloat16)
    )
    in_maps = []
    for c in range(8):
        b, half = c // 2, c % 2
        if half == 0:
            x_loc = x[b, :tkv]
        else:
            x_loc = np.concatenate([x[b, tq:tkv], x[b, :tq]], axis=0)
        m = {"x_loc": np.ascontiguousarray(x_loc.astype(ml_dtypes.bfloat16))}
        m.update(wcast)
        m.update(fp32v)
        in_maps.append(m)
    return in_maps


def kernel(**inputs):
    f = np.asarray
    ln_triv = all(
        bool(np.all(f(inputs[w], dtype=np.float32) == 1.0)) for w in ("ln1_w", "ln2_w")
    ) and all(
        bool(np.all(f(inputs[b], dtype=np.float32) == 0.0)) for b in ("ln1_b", "ln2_b")
    )
    bias_triv = bool(np.all(f(inputs["bv"], dtype=np.float32) == 0.0))
    nc = build(S, ln_trivial=ln_triv, bias_trivial=bias_triv)
    in_maps = make_in_maps(inputs, S)
    res = run_bass_kernel_spmd(nc, in_maps, core_ids=list(range(8)))
    tq = S // 2
    out = np.empty((B, S, H), dtype=np.float32)
    for c in range(8):
        b, half = c // 2, c % 2
        out[b, half * tq : (half + 1) * tq] = res.results[c]["out_loc"]
    return out


# revision 73
# speedup vs baseline: 1.0122x; 1.0122x over previous
"""Trainium2 Bass kernel for a dense pre-norm transformer block.

B, S, H, NH, MLP = 4, 2048, 768, 12, 3072 (fp32 I/O).

Sharding: 8 shards = (batch, seq-half). Each core receives its batch's full
2048-token sequence with its own 1024 query tokens permuted to the front
(attention is permutation-invariant over keys), computes K/V for all 2048
tokens, and Q/attention/MLP for its 1024 query tokens. No collectives.

On-chip: activations are kept feature-major [feature-part, token-free] for
matmuls (weights stationary), token-major for LN/softmax-normalize/residual.
Attention computes scoresT = K @ Q^T per head, exponentiates on ACT
(scale=1/8 folded), then multiplies with a stationary [V | 1/64] so the
softmax denominator accumulates for free in the extra PSUM row; the
normalization happens after a PE transpose back to token-major where the
denominator is a per-partition scalar.

fp8 (e4m3) with DoubleRow perf mode (2 k-tiles per matmul) is used for the
large-contraction GEMMs: Q/K/V/O projections, the probs@V ctx matmul, and
the MLP second linear. Weights are pre-scaled x256 host-side (w std 0.02 is
subnormal in e4m3); the 1/256 dequant rides the PSUM-evacuation ops. Scores
(64-deep contraction) and the MLP first linear (error-critical: feeds gelu)
stay bf16. fp32 accumulation everywhere, fp32 LN/residual spine.

Schedule: the attention kv-loop is software-pipelined (scores run one
kv-pair ahead of the ctx matmul consuming their exp) and the PE slack under
the ACT-bound exp stream is filled with "filler" thunks: the next head
pair's QK projection, the previous head's softmax-normalize, and the
ctx->feature-major transposes feeding the out-projection. The attention
window runs at the ACT exp floor (~100% ACT busy); the MLP phase at ~99% PE.
"""

import sys

if "/opt/trn_rl_repo" not in sys.path:
    sys.path.insert(0, "/opt/trn_rl_repo")

from contextlib import ExitStack

import ml_dtypes
import numpy as np

import concourse.bacc as bacc
import concourse.bass as bass
import concourse.mybir as mybir
import concourse.tile as tile
from concourse.alu_op_type import AluOpType
from concourse.bass_utils import run_bass_kernel_spmd
from concourse.masks import make_identity

B, S, H, NH, MLPD = 4, 2048, 768, 12, 3072
HD = H // NH  # 64
EPS = 1e-6
P = 128
N_H = H // P  # 6
N_M = MLPD // P  # 24
AF = mybir.ActivationFunctionType
BF = mybir.dt.bfloat16
F32 = mybir.dt.float32
F8 = mybir.dt.float8e4
DR = mybir.MatmulPerfMode.DoubleRow
NPF8 = ml_dtypes.float8_e4m3
WS = 256.0  # weight pre-scale (host)
CS = 64.0  # ctx scale via 1/64 ones-column

_BUILD_CACHE = {}


def build(tkv=S, mlp_act="Gelu", ln_trivial=True, bias_trivial=True):
    key = (tkv, mlp_act, ln_trivial, bias_trivial)
    if key in _BUILD_CACHE:
        return _BUILD_CACHE[key]

    tq = tkv // 2
    n_kv = tkv // P  # K/V token tiles
    n_q = tq // P  # query token tiles
    CH = 512 if tq % 512 == 0 else tq  # moving-operand chunk
    n_cq = tq // CH  # query chunks
    n_ckv = tkv // CH  # kv chunks
    n_b = CH // P  # 128-blocks per chunk
    VC = 384  # v-proj output chunk (6 heads)
    n_vc = H // VC  # 2

    nc = bacc.Bacc("TRN2", target_bir_lowering=False, debug=False, num_devices=8)

    x_d = nc.dram_tensor("x_loc", (tkv, H), BF, kind="ExternalInput").ap()
    wq_d = nc.dram_tensor("wq", (H, H), F8, kind="ExternalInput").ap()
    wk_d = nc.dram_tensor("wk", (H, H), F8, kind="ExternalInput").ap()
    wv_d = nc.dram_tensor("wv", (H, H), F8, kind="ExternalInput").ap()
    wo_d = nc.dram_tensor("wo", (H, H), F8, kind="ExternalInput").ap()
    # MLP first linear: leading 4 k-tiles fp8 (x256), trailing 2 bf16
    # (also x256 -- lossless exponent shift -- so the psum shares one scale)
    M1A = 4
    w1a_d = nc.dram_tensor("w1a", (M1A * P, MLPD), F8, kind="ExternalInput").ap()
    w1b_d = nc.dram_tensor("w1b", ((N_H - M1A) * P, MLPD), BF, kind="ExternalInput").ap()
    w2_d = nc.dram_tensor("w2", (MLPD, H), F8, kind="ExternalInput").ap()
    bq_d = nc.dram_tensor("bq", (H,), F32, kind="ExternalInput").ap()
    bk_d = nc.dram_tensor("bk", (H,), F32, kind="ExternalInput").ap()
    bv_d = nc.dram_tensor("bv", (H,), BF, kind="ExternalInput").ap()
    bo_d = nc.dram_tensor("bo", (H,), F32, kind="ExternalInput").ap()
    b1_d = nc.dram_tensor("b1", (MLPD,), F32, kind="ExternalInput").ap()
    b2_d = nc.dram_tensor("b2", (H,), F32, kind="ExternalInput").ap()
    ln1w_d = nc.dram_tensor("ln1_w", (H,), BF, kind="ExternalInput").ap()
    ln1b_d = nc.dram_tensor("ln1_b", (H,), BF, kind="ExternalInput").ap()
    ln2w_d = nc.dram_tensor("ln2_w", (H,), BF, kind="ExternalInput").ap()
    ln2b_d = nc.dram_tensor("ln2_b", (H,), BF, kind="ExternalInput").ap()
    out_d = nc.dram_tensor("out_loc", (tq, H), F32, kind="ExternalOutput").ap()

    def bcast(ap1d):
        return bass.AP(
            tensor=ap1d.tensor, offset=ap1d.offset, ap=[[0, P]] + list(ap1d.ap)
        )

    with tile.TileContext(nc) as tc, ExitStack() as top:
        const = top.enter_context(tc.tile_pool(name="const", bufs=1))
        persist = top.enter_context(tc.tile_pool(name="persist", bufs=1))
        # Top-level PSUM pool: 2 banks shared by transposes + proj accums.
        psum = top.enter_context(tc.tile_pool(name="psum", bufs=1, space="PSUM"))
        toks = top.enter_context(tc.tile_pool(name="toks", bufs=4))
        tmps = top.enter_context(tc.tile_pool(name="tmps", bufs=2))

        # ---- constants ----
        ident = const.tile([P, P], BF)
        make_identity(nc, ident)
        eps_t = const.tile([P, 1], F32)
        nc.vector.memset(eps_t, EPS)
        ln1w_bc = const.tile([P, H], BF)
        nc.gpsimd.dma_start(out=ln1w_bc, in_=bcast(ln1w_d))
        ln1b_bc = const.tile([P, H], BF)
        nc.gpsimd.dma_start(out=ln1b_bc, in_=bcast(ln1b_d))
        ln2w_bc = const.tile([P, H], BF)
        nc.gpsimd.dma_start(out=ln2w_bc, in_=bcast(ln2w_d))
        ln2b_bc = const.tile([P, H], BF)
        nc.gpsimd.dma_start(out=ln2b_bc, in_=bcast(ln2b_d))
        bv_row = const.tile([1, H], BF)
        nc.sync.dma_start(out=bv_row, in_=bv_d[None, :])
        ones_row = const.tile([1, P], BF)
        nc.vector.memset(ones_row, 1.0)
        bq_sb = const.tile([P, N_H], F32)
        nc.sync.dma_start(out=bq_sb, in_=bq_d.rearrange("(t p) -> p t", p=P))
        bk_sb = const.tile([P, N_H], F32)
        nc.sync.dma_start(out=bk_sb, in_=bk_d.rearrange("(t p) -> p t", p=P))
        bo_sb = const.tile([P, N_H], F32)
        nc.sync.dma_start(out=bo_sb, in_=bo_d.rearrange("(t p) -> p t", p=P))
        b1_sb = const.tile([P, N_M], F32)
        nc.sync.dma_start(out=b1_sb, in_=b1_d.rearrange("(t p) -> p t", p=P))
        b2_sb = const.tile([P, N_H], F32)
        nc.sync.dma_start(out=b2_sb, in_=b2_d.rearrange("(t p) -> p t", p=P))
        wo_sb = const.tile([P, N_H, H], F8)

        ctx_tok = persist.tile([P, n_q, H], BF)  # 64*normalized ctx (token-major)
        ctxT = persist.tile([P, N_H, tq], F8)  # feature-major ctx for O-proj

        def ln_tile(x_ap, w_bc, b_bc, out_bf):
            """LayerNorm of one [P, H] tile -> bf16 out (token-major).

            Stats on DVE, the normalize pass on ACT (per-partition
            scale/bias). When the LN weight/bias are the identity (this
            problem's setup), the ACT pass writes the result directly.
            """
            stats = tmps.tile([P, 2, 6], F32, tag="ln_stats", bufs=4)
            for g in range(2):
                nc.vector.bn_stats(out=stats[:, g, :], in_=x_ap[:, g * 384 : (g + 1) * 384])
            mv = tmps.tile([P, 2], F32, tag="ln_mv", bufs=4)
            nc.vector.bn_aggr(out=mv, in_=stats)
            rstd = tmps.tile([P, 1], F32, tag="ln_rstd", bufs=4)
            nc.scalar.activation(out=rstd, in_=mv[:, 1:2], func=AF.Sqrt, bias=eps_t, scale=1.0)
            nc.vector.reciprocal(out=rstd, in_=rstd)
            nmr = tmps.tile([P, 1], F32, tag="ln_nmr", bufs=4)
            nc.vector.scalar_tensor_tensor(
                out=nmr, in0=mv[:, 0:1], scalar=-1.0, in1=rstd,
                op0=AluOpType.mult, op1=AluOpType.mult,
            )
            if ln_trivial:
                nc.scalar.activation(
                    out=out_bf, in_=x_ap, func=AF.Identity, scale=rstd, bias=nmr
                )
            else:
                xh = tmps.tile([P, H], BF, tag="ln_xh", bufs=4)
                nc.scalar.activation(out=xh, in_=x_ap, func=AF.Identity, scale=rstd, bias=nmr)
                nc.vector.tensor_mul(out_bf, xh, w_bc)
                nc.vector.tensor_add(out_bf, out_bf, b_bc)

        def transpose_to(dst_ap, src_ap, rows, cols, copy_eng="scalar"):
            """dst[cols, rows] = src[rows, cols].T (both SBUF; dst may narrow)."""
            pt = psum.tile([P, P], BF, tag="aux", bufs=2)
            nc.tensor.transpose(pt[0:cols, 0:rows], src_ap, ident[0:rows, 0:rows])
            if copy_eng == "scalar":
                nc.scalar.copy(out=dst_ap, in_=pt[0:cols, 0:rows])
            else:
                nc.vector.tensor_copy(out=dst_ap, in_=pt[0:cols, 0:rows])

        # ====== Phase 1-3: LN1, V proj, then per head-pair (QK proj +
        # attention) so PE-dense projection work fills exp-wait gaps. ======
        ACH = min(1024, tq)
        n_ac = tq // ACH
        n_sc = ACH // CH
        with tc.tile_pool(name="qkv_sb", bufs=1) as qkv_sb:
            # Q stored zero-padded per head: head h occupies its 64 rows,
            # the other 64 rows stay zero, so the scores matmul can use the
            # full 128-row kT stationary (FWL) with exact math.
            # Q stored zero-padded per head: head h occupies its 64 rows,
            # the other 64 rows stay zero, so the scores matmul runs the
            # full 128-row stationary (FWL + back-to-back pipelining beat
            # the dense 64-row form by ~2x on hardware).
            qT = qkv_sb.tile([P, NH, tq], BF)
            kT = qkv_sb.tile([P, N_H, tkv], BF)
            # per-head row padded 65->68 so the kv-tile stride (12*68=816B)
            # meets DoubleRow's step%16==0 ldweights requirement
            VP = 68
            vone = qkv_sb.tile([P, n_kv, NH, VP], F8)
            # qT padding rows zeroed on the (otherwise idle) gpsimd engine;
            # finishes long before the first q evac at ~LN1-phase end
            nc.gpsimd.memset(qT, 0.0)

            with tc.tile_pool(name="ln_qkv", bufs=1) as lnp, tc.tile_pool(
                name="attn_sb", bufs=1
            ) as asb:
                xnT = lnp.tile([P, N_H, tkv], F8)
                wq_sb = lnp.tile([P, N_H, H], F8)
                wk_sb = lnp.tile([P, N_H, H], F8)
                wv_sb = lnp.tile([P, N_H, H], F8)
                for i in range(N_H):
                    nc.sync.dma_start(out=wv_sb[:, i, :], in_=wv_d[i * P : (i + 1) * P, :])

                def qk_group(w_sb, b_sb, dstT, hot, c, split_q=False):
                    pk = psum.tile([P, CH], F32, tag="aux", bufs=2)
                    for hit in range(0, N_H, 2):
                        nc.tensor.matmul(
                            pk,
                            w_sb[:, hit : hit + 2, hot * P : (hot + 1) * P],
                            xnT[:, hit : hit + 2, c * CH : (c + 1) * CH],
                            start=(hit == 0), stop=(hit == N_H - 2),
                            perf_mode=DR,
                        )
                    if split_q:
                        nc.vector.tensor_scalar(
                            dstT[0:HD, 2 * hot, c * CH : (c + 1) * CH],
                            pk[0:HD, :], 1.0 / WS,
                            b_sb[:, hot : hot + 1][0:HD],
                            AluOpType.mult, AluOpType.add,
                        )
                        nc.vector.tensor_scalar(
                            dstT[HD:P, 2 * hot + 1, c * CH : (c + 1) * CH],
                            pk[HD:P, :], 1.0 / WS,
                            b_sb[:, hot : hot + 1][HD:P],
                            AluOpType.mult, AluOpType.add,
                        )
                    else:
                        nc.vector.tensor_scalar(
                            dstT[:, hot, c * CH : (c + 1) * CH], pk,
                            1.0 / WS, b_sb[:, hot : hot + 1],
                            AluOpType.mult, AluOpType.add,
                        )

                # LN1 + transpose + V projection, per token tile (keeps PE
                # fed with V matmuls while DVE/ACT chew the next LN). The
                # attention PSUM pool opens only after this loop, so V
                # accumulators and transposes get their own banks here.
                with tc.tile_pool(name="psLN", bufs=1, space="PSUM") as psLN:
                    for t in range(n_kv):
                        x_t = toks.tile([P, H], BF, tag="xtok")
                        nc.sync.dma_start(out=x_t, in_=x_d[t * P : (t + 1) * P, :])
                        xn_bf = tmps.tile([P, H], BF, tag="xn_bf", bufs=4)
                        ln_tile(x_t, ln1w_bc, ln1b_bc, xn_bf)
                        # paired evacuation: two transposes share one strided
                        # copy, halving the per-copy fixed latency. DVE (the
                        # LN1-phase limiter) gets only one of the three.
                        for j2 in range(0, N_H, 2):
                            pt = psum.tile([P, 2, P], BF, tag="aux", bufs=2)
                            nc.tensor.transpose(
                                pt[:, 0, :], xn_bf[:, j2 * P : (j2 + 1) * P], ident
                            )
                            nc.tensor.transpose(
                                pt[:, 1, :], xn_bf[:, (j2 + 1) * P : (j2 + 2) * P], ident
                            )
                            dst = xnT[:, j2 : j2 + 2, t * P : (t + 1) * P]
                            if j2 < 4:
                                nc.scalar.copy(out=dst, in_=pt)
                            else:
                                nc.vector.tensor_copy(out=dst, in_=pt)
                        for c2 in range(n_vc):
                            pv = psLN.tile([P, VC], F32, tag="pv", bufs=4)
                            if not bias_trivial:
                                # bias row via K=1 ones-matmul (bv pre-scaled
                                # x256 host-side)
                                nc.tensor.matmul(
                                    pv, ones_row[:, 0:P],
                                    bv_row[:, c2 * VC : (c2 + 1) * VC],
                                    start=True, stop=False,
                                )
                            for hit in range(0, N_H, 2):
                                nc.tensor.matmul(
                                    pv,
                                    xnT[:, hit : hit + 2, t * P : (t + 1) * P],
                                    wv_sb[:, hit : hit + 2, c2 * VC : (c2 + 1) * VC],
                                    start=(bias_trivial and hit == 0),
                                    stop=(hit == N_H - 2),
                                    perf_mode=DR,
                                )
                            nc.vector.tensor_scalar_mul(
                                vone[:, t, c2 * (VC // HD) : (c2 + 1) * (VC // HD), 0:HD],
                                pv.rearrange("p (h d) -> p h d", d=HD),
                                1.0 / WS,
                            )

                # Q/K/O weights only needed once attention starts; emit
                # their loads after the x/LN traffic so they don't delay it.
                for i in range(N_H):
                    nc.sync.dma_start(out=wq_sb[:, i, :], in_=wq_d[i * P : (i + 1) * P, :])
                    nc.sync.dma_start(out=wk_sb[:, i, :], in_=wk_d[i * P : (i + 1) * P, :])
                    nc.sync.dma_start(out=wo_sb[:, i, :], in_=wo_d[i * P : (i + 1) * P, :])
                # the vone ones-column (tiny) on DVE
                nc.vector.memset(vone[:, :, :, HD : HD + 1], 1.0 / CS)

                def qk_groups(ht):
                    """QK projection for head-pair ht as filler thunks."""
                    for c in range(n_cq):
                        yield lambda c=c: qk_group(wq_sb, bq_sb, qT, ht, c, True)
                    for c in range(n_ckv):
                        yield lambda c=c: qk_group(wk_sb, bk_sb, kT, ht, c)

                psA = []

                def attention_head(h, fillers):
                    """Scores+exp+ctx for head h; pops one filler thunk per
                    kv-pair so the PE queue always has ready work while ACT
                    runs exp. Returns deferred normalize thunks."""
                    ht = h // 2
                    n_p = n_kv // 2
                    norm = []
                    for c in range(n_ac):
                        pctx = psA[0].tile([P, ACH], F32, tag="pctx", bufs=1)

                        def do_scores(kt2, c=c):
                            expair = asb.tile([P, 2, ACH], F8, tag="exp", bufs=4)
                            for j2 in range(2):
                                kt = 2 * kt2 + j2
                                ps = psA[0].tile([P, ACH], F32, tag="psc", bufs=2)
                                for sc in range(n_sc):
                                    nc.tensor.matmul(
                                        ps[:, sc * CH : (sc + 1) * CH],
                                        kT[:, ht, kt * P : (kt + 1) * P],
                                        qT[:, h,
                                           c * ACH + sc * CH : c * ACH + (sc + 1) * CH],
                                        start=True, stop=True,
                                    )
                                nc.scalar.activation(
                                    out=expair[:, j2, :], in_=ps, func=AF.Exp, scale=0.125
                                )
                            return expair

                        def do_ctx(kt2, expair, pctx=pctx):
                            for sc in range(n_sc):
                                nc.tensor.matmul(
                                    pctx[0 : HD + 1, sc * CH : (sc + 1) * CH],
                                    vone[:, 2 * kt2 : 2 * kt2 + 2, h, 0 : HD + 1],
                                    expair[:, :, sc * CH : (sc + 1) * CH],
                                    start=(kt2 == 0), stop=(kt2 == n_p - 1),
                                    perf_mode=DR,
                                )

                        # software pipeline: scores run one kv-pair ahead of
                        # the ctx matmul that consumes its exp; filler thunks
                        # (QK-proj / prev-head normalize) plug the PE slack.
                        prev = do_scores(0)
                        for kt2 in range(1, n_p):
                            cur = do_scores(kt2)
                            do_ctx(kt2 - 1, prev)
                            prev = cur
                            if fillers:
                                fillers.popleft()()
                        do_ctx(n_p - 1, prev)
                        cd = asb.tile([P, ACH], BF, tag="cd", bufs=3)
                        nc.vector.tensor_copy(out=cd[0 : HD + 1, :], in_=pctx[0 : HD + 1, :])

                        def norm_chunk(b4s, c=c, cd=cd):
                            for b4 in b4s:
                                t_tok = c * (ACH // P) + b4
                                pt = psum.tile([P, P], BF, tag="aux", bufs=2)
                                nc.tensor.transpose(
                                    pt[0:P, 0 : HD + 1],
                                    cd[0 : HD + 1, b4 * P : (b4 + 1) * P],
                                    ident[0 : HD + 1, 0 : HD + 1],
                                )
                                rp = tmps.tile([P, 1], F32, tag="rp", bufs=4)
                                nc.vector.reciprocal(rp, pt[:, HD : HD + 1])
                                nc.vector.tensor_scalar_mul(
                                    ctx_tok[:, t_tok, h * HD : (h + 1) * HD],
                                    pt[:, 0:HD],
                                    rp,
                                )

                        nb4 = ACH // P
                        for i0 in range(0, nb4, 2):
                            norm.append(lambda i0=i0: norm_chunk(range(i0, min(i0 + 2, nb4))))
                    return norm

                def ctxT_chunks(ht):
                    """Transpose the normalized pair-ht ctx into feature-major
                    ctxT (O-proj stationary input) as filler thunks. Two
                    transposes share one contiguous evacuation copy."""
                    def one(t0):
                        pt = psum.tile([P, 2, P], BF, tag="aux", bufs=2)
                        for i in range(2):
                            nc.tensor.transpose(
                                pt[:, i, :],
                                ctx_tok[:, t0 + i, ht * P : (ht + 1) * P],
                                ident,
                            )
                        nc.vector.tensor_copy(
                            out=ctxT[:, ht, t0 * P : (t0 + 2) * P], in_=pt
                        )
                    for t0 in range(0, n_q, 2):
                        yield lambda t0=t0: one(t0)

                from collections import deque

                with tc.tile_pool(name="psA", bufs=1, space="PSUM") as psA_:
                    psA.append(psA_)
                    # pair 0's QK-proj has no attention to hide in: run now.
                    for f in qk_groups(0):
                        f()
                    carried = deque()
                    for ht in range(N_H):
                        fillers = deque()
                        if ht + 1 < N_H:
                            fillers.extend(qk_groups(ht + 1))
                        fillers.extend(carried)
                        carried = deque()
                        norm0 = attention_head(2 * ht, fillers)
                        # the even head's normalize can fill the odd head's
                        # slots within the same pair
                        fillers.extend(norm0)
                        norm1 = attention_head(2 * ht + 1, fillers)
                        # leftover fillers (incl. prev pair's normalize) now
                        while fillers:
                            fillers.popleft()()
                        carried.extend(norm1)
                        carried.extend(ctxT_chunks(ht))
                    while carried:
                        carried.popleft()()

        # ========== Phase 4-6 ==========
        with tc.tile_pool(name="late", bufs=1) as late:
            x1_sb = late.tile([P, n_q, H], F32)  # attn-block out (token-major)
            # prefetch the residual x tiles so the out-proj phase never
            # waits on DMA. On the gpsimd queue: the wait for attention-phase
            # SBUF to free must not block the sync queue (w1/w2 loads).
            xr_all = late.tile([P, n_q, H], BF)
            for t in range(n_q):
                nc.gpsimd.dma_start(out=xr_all[:, t, :], in_=x_d[t * P : (t + 1) * P, :])

            # ---- out-proj + residual + LN2 (chunked), then MLP ----
            # The MLP pool opens before the out-proj loop so each chunk's
            # LN2 pass runs right after its residual: MLP chunk 0 is ready
            # while chunk 1's out-projection still runs. (The w1/w2 loads
            # block the sync queue on SBUF-free, but nothing else needs it
            # until the output DMAs.)
            with tc.tile_pool(name="mlp_sb", bufs=1) as mp, tc.tile_pool(
                name="ps6", bufs=1, space="PSUM"
            ) as ps6:
                xn2T8 = mp.tile([P, M1A, tq], F8)
                xn2T = mp.tile([P, N_H - M1A, tq], BF)
                w1a_sb = mp.tile([P, M1A, MLPD], F8)
                w1b_sb = mp.tile([P, N_H - M1A, MLPD], BF)
                w2_sb = mp.tile([P, N_M, H], F8)
                h1c = mp.tile([P, N_M, CH], F8)
                y2T = mp.tile([P, N_H, CH], BF)
                for i in range(M1A):
                    nc.sync.dma_start(out=w1a_sb[:, i, :], in_=w1a_d[i * P : (i + 1) * P, :])
                for i in range(N_H - M1A):
                    nc.sync.dma_start(out=w1b_sb[:, i, :], in_=w1b_d[i * P : (i + 1) * P, :])
                for i in range(N_M):
                    nc.sync.dma_start(out=w2_sb[:, i, :], in_=w2_d[i * P : (i + 1) * P, :])

                def ln2_tile(t):
                    xn2_bf = tmps.tile([P, H], BF, tag="xn_bf", bufs=4)
                    ln_tile(x1_sb[:, t, :], ln2w_bc, ln2b_bc, xn2_bf)
                    # paired evacuation (see xnT): pairs (0,1),(2,3) land in
                    # the fp8 tile, (4,5) in the bf16 tile
                    for j2 in range(0, N_H, 2):
                        pt = psum.tile([P, 2, P], BF, tag="aux", bufs=2)
                        nc.tensor.transpose(
                            pt[:, 0, :], xn2_bf[:, j2 * P : (j2 + 1) * P], ident
                        )
                        nc.tensor.transpose(
                            pt[:, 1, :], xn2_bf[:, (j2 + 1) * P : (j2 + 2) * P], ident
                        )
                        dst = (
                            xn2T8[:, j2 : j2 + 2, t * P : (t + 1) * P]
                            if j2 < M1A
                            else xn2T[:, 0:2, t * P : (t + 1) * P]
                        )
                        nc.vector.tensor_copy(out=dst, in_=pt)

                uT = mp.tile([P, N_H, tq], BF)

                def o_chunk(c):
                    for hot in range(N_H):
                        pu = psum.tile([P, CH], F32, tag="aux", bufs=2)
                        for hit in range(0, N_H, 2):
                            nc.tensor.matmul(
                                pu,
                                wo_sb[:, hit : hit + 2, hot * P : (hot + 1) * P],
                                ctxT[:, hit : hit + 2, c * CH : (c + 1) * CH],
                                start=(hit == 0), stop=(hit == N_H - 2),
                                perf_mode=DR,
                            )
                        nc.vector.tensor_scalar(
                            uT[:, hot, c * CH : (c + 1) * CH], pu,
                            1.0 / (WS * CS), bo_sb[:, hot : hot + 1],
                            AluOpType.mult, AluOpType.add,
                        )

                def res_ln2(c):
                    for t in range(c * n_b, (c + 1) * n_b):
                        for j2 in range(0, N_H, 2):
                            pt = psum.tile([P, 2, P], BF, tag="aux", bufs=2)
                            nc.tensor.transpose(
                                pt[:, 0, :], uT[:, j2, t * P : (t + 1) * P], ident,
                            )
                            nc.tensor.transpose(
                                pt[:, 1, :], uT[:, j2 + 1, t * P : (t + 1) * P], ident,
                            )
                            nc.vector.tensor_add(
                                x1_sb[:, t, j2 * P : (j2 + 2) * P],
                                pt,
                                xr_all[:, t, j2 * P : (j2 + 2) * P],
                            )
                    for t in range(c * n_b, (c + 1) * n_b):
                        ln2_tile(t)

                def mlp_ph(c):
                    for mt in range(N_M):
                        ph = ps6.tile([P, CH], F32, tag="pmm", bufs=4)
                        for hit in range(0, M1A, 2):
                            nc.tensor.matmul(
                                ph,
                                w1a_sb[:, hit : hit + 2, mt * P : (mt + 1) * P],
                                xn2T8[:, hit : hit + 2, c * CH : (c + 1) * CH],
                                start=(hit == 0), stop=False,
                                perf_mode=DR,
                            )
                        for hit in range(N_H - M1A):
                            nc.tensor.matmul(
                                ph,
                                w1b_sb[:, hit, mt * P : (mt + 1) * P],
                                xn2T[:, hit, c * CH : (c + 1) * CH],
                                start=False, stop=(hit == N_H - M1A - 1),
                            )
                        nc.scalar.activation(
                            out=h1c[:, mt, :], in_=ph,
                            func=getattr(AF, mlp_act), bias=b1_sb[:, mt : mt + 1],
                            scale=1.0 / WS,
                        )

                def mlp_tail(c):
                    for hot in range(N_H):
                        py = ps6.tile([P, CH], F32, tag="pmm", bufs=4)
                        for mt in range(0, N_M, 2):
                            nc.tensor.matmul(
                                py,
                                w2_sb[:, mt : mt + 2, hot * P : (hot + 1) * P],
                                h1c[:, mt : mt + 2, :],
                                start=(mt == 0), stop=(mt == N_M - 2),
                                perf_mode=DR,
                            )
                        nc.vector.tensor_scalar(
                            y2T[:, hot, :], py, 1.0 / WS, b2_sb[:, hot : hot + 1],
                            AluOpType.mult, AluOpType.add,
                        )
                    for b4 in range(n_b):
                        t = c * n_b + b4
                        outt = toks.tile([P, H], F32, tag="xtok")
                        for j2 in range(0, N_H, 2):
                            pt = psum.tile([P, 2, P], BF, tag="aux", bufs=2)
                            nc.tensor.transpose(
                                pt[:, 0, :], y2T[:, j2, b4 * P : (b4 + 1) * P], ident,
                            )
                            nc.tensor.transpose(
                                pt[:, 1, :], y2T[:, j2 + 1, b4 * P : (b4 + 1) * P], ident,
                            )
                            nc.vector.tensor_add(
                                outt[:, j2 * P : (j2 + 2) * P],
                                pt,
                                x1_sb[:, t, j2 * P : (j2 + 2) * P],
                            )
                        # alternate DMA queues so the output drain overlaps
                        eng = nc.sync if t % 2 == 0 else nc.gpsimd
                        eng.dma_start(out=out_d[t * P : (t + 1) * P, :], in_=outt)

                # interleaved emission: both out-proj chunks back-to-back
                # (36 DR matmuls keep PE dense while DVE evacuates), then
                # MLP chunk 0's first-linear groups fill the PE dip while
                # chunk 1's residual/LN2 chain resolves.
                o_chunk(0)
                o_chunk(1)
                res_ln2(0)
                mlp_ph(0)
                res_ln2(1)
                mlp_tail(0)
                mlp_ph(1)
                mlp_tail(1)

    nc.compile()
    _BUILD_CACHE[key] = nc
    return nc


def make_in_maps(inputs, tkv=S):
    """Build the 8 per-core input maps from full inputs."""
    f = np.asarray
    x = f(inputs["x"], dtype=np.float32)
    tq = tkv // 2
    wcast = {
        n: np.ascontiguousarray((f(inputs[n], dtype=np.float32) * WS).astype(NPF8))
        for n in ["wq", "wk", "wv", "wo", "w2"]
    }
    w1f = f(inputs["w1"], dtype=np.float32) * WS
    wcast["w1a"] = np.ascontiguousarray(w1f[: 4 * P].astype(NPF8))
    wcast["w1b"] = np.ascontiguousarray(w1f[4 * P :].astype(ml_dtypes.bfloat16))
    fp32v = {
        n: np.ascontiguousarray(f(inputs[n], dtype=np.float32))
        for n in ["bq", "bk", "bo", "b1", "b2"]
    }
    for n in ["ln1_w", "ln1_b", "ln2_w", "ln2_b"]:
        fp32v[n] = np.ascontiguousarray(f(inputs[n]).astype(ml_dtypes.bfloat16))
    # bv rides a ones-matmul row inside the x256-scaled V psum
    fp32v["bv"] = np.ascontiguousarray(
        (f(inputs["bv"], dtype=np.float32) * WS).astype(ml_dtypes.bfloat16)
    )
    in_maps = []
    for c in range(8):
        b, half = c // 2, c % 2
        if half == 0:
            x_loc = x[b, :tkv]
        else:
            x_loc = np.concatenate([x[b, tq:tkv], x[b, :tq]], axis=0)
        m = {"x_loc": np.ascontiguousarray(x_loc.astype(ml_dtypes.bfloat16))}
        m.update(wcast)
        m.update(fp32v)
        in_maps.append(m)
    return in_maps


def kernel(**inputs):
    f = np.asarray
    ln_triv = all(
        bool(np.all(f(inputs[w], dtype=np.float32) == 1.0)) for w in ("ln1_w", "ln2_w")
    ) and all(
        bool(np.all(f(inputs[b], dtype=np.float32) == 0.0)) for b in ("ln1_b", "ln2_b")
    )
    bias_triv = bool(np.all(f(inputs["bv"], dtype=np.float32) == 0.0))
    nc = build(S, ln_trivial=ln_triv, bias_trivial=bias_triv)
    in_maps = make_in_maps(inputs, S)
    res = run_bass_kernel_spmd(nc, in_maps, core_ids=list(range(8)))
    tq = S // 2
    out = np.empty((B, S, H), dtype=np.float32)
    for c in range(8):
        b, half = c // 2, c % 2
        out[b, half * tq : (half + 1) * tq] = res.results[c]["out_loc"]
    return out


# revision 75
# speedup vs baseline: 1.0331x; 1.0207x over previous
"""Trainium2 Bass kernel for a dense pre-norm transformer block.

B, S, H, NH, MLP = 4, 2048, 768, 12, 3072 (fp32 I/O).

Sharding: 8 shards = (batch, seq-half). Each core receives its batch's full
2048-token sequence with its own 1024 query tokens permuted to the front
(attention is permutation-invariant over keys), computes K/V for all 2048
tokens, and Q/attention/MLP for its 1024 query tokens. No collectives.

On-chip: activations are kept feature-major [feature-part, token-free] for
matmuls (weights stationary), token-major for LN/softmax-normalize/residual.
Attention computes scoresT = K @ Q^T per head, exponentiates on ACT
(scale=1/8 folded), then multiplies with a stationary [V | 1/64] so the
softmax denominator accumulates for free in the extra PSUM row; the
normalization happens after a PE transpose back to token-major where the
denominator is a per-partition scalar.

fp8 (e4m3) with DoubleRow perf mode (2 k-tiles per matmul) is used for the
large-contraction GEMMs: Q/K/V/O projections, the probs@V ctx matmul, and
the MLP second linear. Weights are pre-scaled x256 host-side (w std 0.02 is
subnormal in e4m3); the 1/256 dequant rides the PSUM-evacuation ops. Scores
(64-deep contraction) and the MLP first linear (error-critical: feeds gelu)
stay bf16. fp32 accumulation everywhere, fp32 LN/residual spine.

Schedule: the attention kv-loop is software-pipelined (scores run one
kv-pair ahead of the ctx matmul consuming their exp) and the PE slack under
the ACT-bound exp stream is filled with "filler" thunks: the next head
pair's QK projection, the previous head's softmax-normalize, and the
ctx->feature-major transposes feeding the out-projection. The attention
window runs at the ACT exp floor (~100% ACT busy); the MLP phase at ~99% PE.
"""

import sys

if "/opt/trn_rl_repo" not in sys.path:
    sys.path.insert(0, "/opt/trn_rl_repo")

from contextlib import ExitStack

import ml_dtypes
import numpy as np

import concourse.bacc as bacc
import concourse.bass as bass
import concourse.mybir as mybir
import concourse.tile as tile
from concourse.alu_op_type import AluOpType
from concourse.bass_utils import run_bass_kernel_spmd
from concourse.masks import make_identity

B, S, H, NH, MLPD = 4, 2048, 768, 12, 3072
HD = H // NH  # 64
EPS = 1e-6
P = 128
N_H = H // P  # 6
N_M = MLPD // P  # 24
AF = mybir.ActivationFunctionType
BF = mybir.dt.bfloat16
F32 = mybir.dt.float32
F8 = mybir.dt.float8e4
DR = mybir.MatmulPerfMode.DoubleRow
NPF8 = ml_dtypes.float8_e4m3
WS = 256.0  # weight pre-scale (host)
CS = 64.0  # ctx scale via 1/64 ones-column

_BUILD_CACHE = {}


def build(tkv=S, mlp_act="Gelu", ln_trivial=True, bias_trivial=True):
    key = (tkv, mlp_act, ln_trivial, bias_trivial)
    if key in _BUILD_CACHE:
        return _BUILD_CACHE[key]

    tq = tkv // 2
    n_kv = tkv // P  # K/V token tiles
    n_q = tq // P  # query token tiles
    CH = 512 if tq % 512 == 0 else tq  # moving-operand chunk
    n_cq = tq // CH  # query chunks
    n_ckv = tkv // CH  # kv chunks
    n_b = CH // P  # 128-blocks per chunk
    VC = 384  # v-proj output chunk (6 heads)
    n_vc = H // VC  # 2

    nc = bacc.Bacc("TRN2", target_bir_lowering=False, debug=False, num_devices=8)

    x_d = nc.dram_tensor("x_loc", (tkv, H), BF, kind="ExternalInput").ap()
    wq_d = nc.dram_tensor("wq", (H, H), F8, kind="ExternalInput").ap()
    wk_d = nc.dram_tensor("wk", (H, H), F8, kind="ExternalInput").ap()
    wv_d = nc.dram_tensor("wv", (H, H), F8, kind="ExternalInput").ap()
    wo_d = nc.dram_tensor("wo", (H, H), F8, kind="ExternalInput").ap()
    # MLP first linear: leading 4 k-tiles fp8 (x256), trailing 2 bf16
    # (also x256 -- lossless exponent shift -- so the psum shares one scale)
    M1A = 4
    w1a_d = nc.dram_tensor("w1a", (M1A * P, MLPD), F8, kind="ExternalInput").ap()
    w1b_d = nc.dram_tensor("w1b", ((N_H - M1A) * P, MLPD), BF, kind="ExternalInput").ap()
    w2_d = nc.dram_tensor("w2", (MLPD, H), F8, kind="ExternalInput").ap()
    bq_d = nc.dram_tensor("bq", (H,), F32, kind="ExternalInput").ap()
    bk_d = nc.dram_tensor("bk", (H,), F32, kind="ExternalInput").ap()
    bv_d = nc.dram_tensor("bv", (H,), BF, kind="ExternalInput").ap()
    bo_d = nc.dram_tensor("bo", (H,), F32, kind="ExternalInput").ap()
    b1_d = nc.dram_tensor("b1", (MLPD,), F32, kind="ExternalInput").ap()
    b2_d = nc.dram_tensor("b2", (H,), F32, kind="ExternalInput").ap()
    ln1w_d = nc.dram_tensor("ln1_w", (H,), BF, kind="ExternalInput").ap()
    ln1b_d = nc.dram_tensor("ln1_b", (H,), BF, kind="ExternalInput").ap()
    ln2w_d = nc.dram_tensor("ln2_w", (H,), BF, kind="ExternalInput").ap()
    ln2b_d = nc.dram_tensor("ln2_b", (H,), BF, kind="ExternalInput").ap()
    out_d = nc.dram_tensor("out_loc", (tq, H), F32, kind="ExternalOutput").ap()

    def bcast(ap1d):
        return bass.AP(
            tensor=ap1d.tensor, offset=ap1d.offset, ap=[[0, P]] + list(ap1d.ap)
        )

    with tile.TileContext(nc) as tc, ExitStack() as top:
        const = top.enter_context(tc.tile_pool(name="const", bufs=1))
        persist = top.enter_context(tc.tile_pool(name="persist", bufs=1))
        # Top-level PSUM pool: 2 banks shared by transposes + proj accums.
        psum = top.enter_context(tc.tile_pool(name="psum", bufs=1, space="PSUM"))
        toks = top.enter_context(tc.tile_pool(name="toks", bufs=8))
        tmps = top.enter_context(tc.tile_pool(name="tmps", bufs=2))

        # ---- constants ----
        ident = const.tile([P, P], BF)
        make_identity(nc, ident)
        eps_t = const.tile([P, 1], F32)
        nc.vector.memset(eps_t, EPS)
        ln1w_bc = const.tile([P, H], BF)
        nc.gpsimd.dma_start(out=ln1w_bc, in_=bcast(ln1w_d))
        ln1b_bc = const.tile([P, H], BF)
        nc.gpsimd.dma_start(out=ln1b_bc, in_=bcast(ln1b_d))
        ln2w_bc = const.tile([P, H], BF)
        nc.gpsimd.dma_start(out=ln2w_bc, in_=bcast(ln2w_d))
        ln2b_bc = const.tile([P, H], BF)
        nc.gpsimd.dma_start(out=ln2b_bc, in_=bcast(ln2b_d))
        bv_row = const.tile([1, H], BF)
        nc.sync.dma_start(out=bv_row, in_=bv_d[None, :])
        ones_row = const.tile([1, P], BF)
        nc.vector.memset(ones_row, 1.0)
        bq_sb = const.tile([P, N_H], F32)
        nc.sync.dma_start(out=bq_sb, in_=bq_d.rearrange("(t p) -> p t", p=P))
        bk_sb = const.tile([P, N_H], F32)
        nc.sync.dma_start(out=bk_sb, in_=bk_d.rearrange("(t p) -> p t", p=P))
        bo_sb = const.tile([P, N_H], F32)
        nc.sync.dma_start(out=bo_sb, in_=bo_d.rearrange("(t p) -> p t", p=P))
        b1_sb = const.tile([P, N_M], F32)
        nc.sync.dma_start(out=b1_sb, in_=b1_d.rearrange("(t p) -> p t", p=P))
        b2_sb = const.tile([P, N_H], F32)
        nc.sync.dma_start(out=b2_sb, in_=b2_d.rearrange("(t p) -> p t", p=P))
        wo_sb = const.tile([P, N_H, H], F8)

        ctx_tok = persist.tile([P, n_q, H], BF)  # 64*normalized ctx (token-major)
        ctxT = persist.tile([P, N_H, tq], F8)  # feature-major ctx for O-proj

        def ln_tile(x_ap, w_bc, b_bc, out_bf):
            """LayerNorm of one [P, H] tile -> bf16 out (token-major).

            Stats on DVE, the normalize pass on ACT (per-partition
            scale/bias). When the LN weight/bias are the identity (this
            problem's setup), the ACT pass writes the result directly.
            """
            stats = tmps.tile([P, 2, 6], F32, tag="ln_stats", bufs=6)
            for g in range(2):
                nc.vector.bn_stats(out=stats[:, g, :], in_=x_ap[:, g * 384 : (g + 1) * 384])
            mv = tmps.tile([P, 2], F32, tag="ln_mv", bufs=6)
            nc.vector.bn_aggr(out=mv, in_=stats)
            rstd = tmps.tile([P, 1], F32, tag="ln_rstd", bufs=6)
            nc.scalar.activation(out=rstd, in_=mv[:, 1:2], func=AF.Sqrt, bias=eps_t, scale=1.0)
            nc.vector.reciprocal(out=rstd, in_=rstd)
            nmr = tmps.tile([P, 1], F32, tag="ln_nmr", bufs=6)
            nc.vector.scalar_tensor_tensor(
                out=nmr, in0=mv[:, 0:1], scalar=-1.0, in1=rstd,
                op0=AluOpType.mult, op1=AluOpType.mult,
            )
            if ln_trivial:
                nc.scalar.activation(
                    out=out_bf, in_=x_ap, func=AF.Identity, scale=rstd, bias=nmr
                )
            else:
                xh = tmps.tile([P, H], BF, tag="ln_xh", bufs=4)
                nc.scalar.activation(out=xh, in_=x_ap, func=AF.Identity, scale=rstd, bias=nmr)
                nc.vector.tensor_mul(out_bf, xh, w_bc)
                nc.vector.tensor_add(out_bf, out_bf, b_bc)

        def transpose_to(dst_ap, src_ap, rows, cols, copy_eng="scalar"):
            """dst[cols, rows] = src[rows, cols].T (both SBUF; dst may narrow)."""
            pt = psum.tile([P, P], BF, tag="aux", bufs=2)
            nc.tensor.transpose(pt[0:cols, 0:rows], src_ap, ident[0:rows, 0:rows])
            if copy_eng == "scalar":
                nc.scalar.copy(out=dst_ap, in_=pt[0:cols, 0:rows])
            else:
                nc.vector.tensor_copy(out=dst_ap, in_=pt[0:cols, 0:rows])

        # ====== Phase 1-3: LN1, V proj, then per head-pair (QK proj +
        # attention) so PE-dense projection work fills exp-wait gaps. ======
        ACH = min(1024, tq)
        n_ac = tq // ACH
        n_sc = ACH // CH
        with tc.tile_pool(name="qkv_sb", bufs=1) as qkv_sb:
            # Q stored zero-padded per head: head h occupies its 64 rows,
            # the other 64 rows stay zero, so the scores matmul can use the
            # full 128-row kT stationary (FWL) with exact math.
            # Q stored zero-padded per head: head h occupies its 64 rows,
            # the other 64 rows stay zero, so the scores matmul runs the
            # full 128-row stationary (FWL + back-to-back pipelining beat
            # the dense 64-row form by ~2x on hardware).
            qT = qkv_sb.tile([P, NH, tq], BF)
            kT = qkv_sb.tile([P, N_H, tkv], BF)
            # per-head row padded 65->68 so the kv-tile stride (12*68=816B)
            # meets DoubleRow's step%16==0 ldweights requirement
            VP = 68
            vone = qkv_sb.tile([P, n_kv, NH, VP], F8)
            # qT padding rows zeroed on the (otherwise idle) gpsimd engine;
            # finishes long before the first q evac at ~LN1-phase end
            nc.gpsimd.memset(qT, 0.0)

            with tc.tile_pool(name="ln_qkv", bufs=1) as lnp, tc.tile_pool(
                name="attn_sb", bufs=1
            ) as asb:
                xnT = lnp.tile([P, N_H, tkv], F8)
                wq_sb = lnp.tile([P, N_H, H], F8)
                wk_sb = lnp.tile([P, N_H, H], F8)
                wv_sb = lnp.tile([P, N_H, H], F8)
                for i in range(N_H):
                    nc.sync.dma_start(out=wv_sb[:, i, :], in_=wv_d[i * P : (i + 1) * P, :])

                def qk_group(w_sb, b_sb, dstT, hot, c, split_q=False):
                    pk = psum.tile([P, CH], F32, tag="aux", bufs=2)
                    for hit in range(0, N_H, 2):
                        nc.tensor.matmul(
                            pk,
                            w_sb[:, hit : hit + 2, hot * P : (hot + 1) * P],
                            xnT[:, hit : hit + 2, c * CH : (c + 1) * CH],
                            start=(hit == 0), stop=(hit == N_H - 2),
                            perf_mode=DR,
                        )
                    if split_q:
                        nc.vector.tensor_scalar(
                            dstT[0:HD, 2 * hot, c * CH : (c + 1) * CH],
                            pk[0:HD, :], 1.0 / WS,
                            b_sb[:, hot : hot + 1][0:HD],
                            AluOpType.mult, AluOpType.add,
                        )
                        nc.vector.tensor_scalar(
                            dstT[HD:P, 2 * hot + 1, c * CH : (c + 1) * CH],
                            pk[HD:P, :], 1.0 / WS,
                            b_sb[:, hot : hot + 1][HD:P],
                            AluOpType.mult, AluOpType.add,
                        )
                    else:
                        nc.vector.tensor_scalar(
                            dstT[:, hot, c * CH : (c + 1) * CH], pk,
                            1.0 / WS, b_sb[:, hot : hot + 1],
                            AluOpType.mult, AluOpType.add,
                        )

                # LN1 + transpose + V projection, per token tile (keeps PE
                # fed with V matmuls while DVE/ACT chew the next LN). The
                # attention PSUM pool opens only after this loop, so V
                # accumulators and transposes get their own banks here.
                with tc.tile_pool(name="psLN", bufs=1, space="PSUM") as psLN:
                    for t in range(n_kv):
                        x_t = toks.tile([P, H], BF, tag="xtok")
                        nc.sync.dma_start(out=x_t, in_=x_d[t * P : (t + 1) * P, :])
                        xn_bf = tmps.tile([P, H], BF, tag="xn_bf", bufs=6)
                        ln_tile(x_t, ln1w_bc, ln1b_bc, xn_bf)
                        # paired evacuation: two transposes share one strided
                        # copy, halving the per-copy fixed latency. DVE (the
                        # LN1-phase limiter) gets only one of the three.
                        for j2 in range(0, N_H, 2):
                            pt = psum.tile([P, 2, P], BF, tag="aux", bufs=2)
                            nc.tensor.transpose(
                                pt[:, 0, :], xn_bf[:, j2 * P : (j2 + 1) * P], ident
                            )
                            nc.tensor.transpose(
                                pt[:, 1, :], xn_bf[:, (j2 + 1) * P : (j2 + 2) * P], ident
                            )
                            dst = xnT[:, j2 : j2 + 2, t * P : (t + 1) * P]
                            if j2 < 4:
                                nc.scalar.copy(out=dst, in_=pt)
                            else:
                                nc.vector.tensor_copy(out=dst, in_=pt)
                        for c2 in range(n_vc):
                            pv = psLN.tile([P, VC], F32, tag="pv", bufs=4)
                            if not bias_trivial:
                                # bias row via K=1 ones-matmul (bv pre-scaled
                                # x256 host-side)
                                nc.tensor.matmul(
                                    pv, ones_row[:, 0:P],
                                    bv_row[:, c2 * VC : (c2 + 1) * VC],
                                    start=True, stop=False,
                                )
                            for hit in range(0, N_H, 2):
                                nc.tensor.matmul(
                                    pv,
                                    xnT[:, hit : hit + 2, t * P : (t + 1) * P],
                                    wv_sb[:, hit : hit + 2, c2 * VC : (c2 + 1) * VC],
                                    start=(bias_trivial and hit == 0),
                                    stop=(hit == N_H - 2),
                                    perf_mode=DR,
                                )
                            nc.vector.tensor_scalar_mul(
                                vone[:, t, c2 * (VC // HD) : (c2 + 1) * (VC // HD), 0:HD],
                                pv.rearrange("p (h d) -> p h d", d=HD),
                                1.0 / WS,
                            )

                # Q/K/O weights only needed once attention starts; emit
                # their loads after the x/LN traffic so they don't delay it.
                for i in range(N_H):
                    nc.sync.dma_start(out=wq_sb[:, i, :], in_=wq_d[i * P : (i + 1) * P, :])
                    nc.sync.dma_start(out=wk_sb[:, i, :], in_=wk_d[i * P : (i + 1) * P, :])
                    nc.sync.dma_start(out=wo_sb[:, i, :], in_=wo_d[i * P : (i + 1) * P, :])
                # the vone ones-column (tiny) on DVE
                nc.vector.memset(vone[:, :, :, HD : HD + 1], 1.0 / CS)

                def qk_groups(ht):
                    """QK projection for head-pair ht as filler thunks."""
                    for c in range(n_cq):
                        yield lambda c=c: qk_group(wq_sb, bq_sb, qT, ht, c, True)
                    for c in range(n_ckv):
                        yield lambda c=c: qk_group(wk_sb, bk_sb, kT, ht, c)

                psA = []

                def attention_head(h, fillers):
                    """Scores+exp+ctx for head h; pops one filler thunk per
                    kv-pair so the PE queue always has ready work while ACT
                    runs exp. Returns deferred normalize thunks."""
                    ht = h // 2
                    n_p = n_kv // 2
                    norm = []
                    for c in range(n_ac):
                        pctx = psA[0].tile([P, ACH], F32, tag="pctx", bufs=1)

                        def do_scores(kt2, c=c):
                            expair = asb.tile([P, 2, ACH], F8, tag="exp", bufs=4)
                            for j2 in range(2):
                                kt = 2 * kt2 + j2
                                ps = psA[0].tile([P, ACH], F32, tag="psc", bufs=2)
                                for sc in range(n_sc):
                                    nc.tensor.matmul(
                                        ps[:, sc * CH : (sc + 1) * CH],
                                        kT[:, ht, kt * P : (kt + 1) * P],
                                        qT[:, h,
                                           c * ACH + sc * CH : c * ACH + (sc + 1) * CH],
                                        start=True, stop=True,
                                    )
                                nc.scalar.activation(
                                    out=expair[:, j2, :], in_=ps, func=AF.Exp, scale=0.125
                                )
                            return expair

                        def do_ctx(kt2, expair, pctx=pctx):
                            for sc in range(n_sc):
                                nc.tensor.matmul(
                                    pctx[0 : HD + 1, sc * CH : (sc + 1) * CH],
                                    vone[:, 2 * kt2 : 2 * kt2 + 2, h, 0 : HD + 1],
                                    expair[:, :, sc * CH : (sc + 1) * CH],
                                    start=(kt2 == 0), stop=(kt2 == n_p - 1),
                                    perf_mode=DR,
                                )

                        # software pipeline: scores run one kv-pair ahead of
                        # the ctx matmul that consumes its exp; filler thunks
                        # (QK-proj / prev-head normalize) plug the PE slack.
                        prev = do_scores(0)
                        for kt2 in range(1, n_p):
                            cur = do_scores(kt2)
                            do_ctx(kt2 - 1, prev)
                            prev = cur
                            if fillers:
                                fillers.popleft()()
                        do_ctx(n_p - 1, prev)
                        cd = asb.tile([P, ACH], BF, tag="cd", bufs=3)
                        nc.vector.tensor_copy(out=cd[0 : HD + 1, :], in_=pctx[0 : HD + 1, :])

                        def norm_chunk(b4s, c=c, cd=cd):
                            for b4 in b4s:
                                t_tok = c * (ACH // P) + b4
                                pt = psum.tile([P, P], BF, tag="aux", bufs=2)
                                nc.tensor.transpose(
                                    pt[0:P, 0 : HD + 1],
                                    cd[0 : HD + 1, b4 * P : (b4 + 1) * P],
                                    ident[0 : HD + 1, 0 : HD + 1],
                                )
                                rp = tmps.tile([P, 1], F32, tag="rp", bufs=4)
                                nc.vector.reciprocal(rp, pt[:, HD : HD + 1])
                                nc.vector.tensor_scalar_mul(
                                    ctx_tok[:, t_tok, h * HD : (h + 1) * HD],
                                    pt[:, 0:HD],
                                    rp,
                                )

                        nb4 = ACH // P
                        for i0 in range(0, nb4, 2):
                            norm.append(lambda i0=i0: norm_chunk(range(i0, min(i0 + 2, nb4))))
                    return norm

                def ctxT_chunks(ht):
                    """Transpose the normalized pair-ht ctx into feature-major
                    ctxT (O-proj stationary input) as filler thunks. Two
                    transposes share one contiguous evacuation copy."""
                    def one(t0):
                        pt = psum.tile([P, 2, P], BF, tag="aux", bufs=2)
                        for i in range(2):
                            nc.tensor.transpose(
                                pt[:, i, :],
                                ctx_tok[:, t0 + i, ht * P : (ht + 1) * P],
                                ident,
                            )
                        nc.vector.tensor_copy(
                            out=ctxT[:, ht, t0 * P : (t0 + 2) * P], in_=pt
                        )
                    for t0 in range(0, n_q, 2):
                        yield lambda t0=t0: one(t0)

                from collections import deque

                with tc.tile_pool(name="psA", bufs=1, space="PSUM") as psA_:
                    psA.append(psA_)
                    # pair 0's QK-proj has no attention to hide in: run now.
                    for f in qk_groups(0):
                        f()
                    carried = deque()
                    for ht in range(N_H):
                        fillers = deque()
                        if ht + 1 < N_H:
                            fillers.extend(qk_groups(ht + 1))
                        fillers.extend(carried)
                        carried = deque()
                        norm0 = attention_head(2 * ht, fillers)
                        # the even head's normalize can fill the odd head's
                        # slots within the same pair
                        fillers.extend(norm0)
                        norm1 = attention_head(2 * ht + 1, fillers)
                        # leftover fillers (incl. prev pair's normalize) now
                        while fillers:
                            fillers.popleft()()
                        carried.extend(norm1)
                        carried.extend(ctxT_chunks(ht))
                    while carried:
                        carried.popleft()()

        # ========== Phase 4-6 ==========
        with tc.tile_pool(name="late", bufs=1) as late:
            x1_sb = late.tile([P, n_q, H], F32)  # attn-block out (token-major)
            # prefetch the residual x tiles so the out-proj phase never
            # waits on DMA. On the gpsimd queue: the wait for attention-phase
            # SBUF to free must not block the sync queue (w1/w2 loads).
            xr_all = late.tile([P, n_q, H], BF)
            for t in range(n_q):
                nc.gpsimd.dma_start(out=xr_all[:, t, :], in_=x_d[t * P : (t + 1) * P, :])

            # ---- out-proj + residual + LN2 (chunked), then MLP ----
            # The MLP pool opens before the out-proj loop so each chunk's
            # LN2 pass runs right after its residual: MLP chunk 0 is ready
            # while chunk 1's out-projection still runs. (The w1/w2 loads
            # block the sync queue on SBUF-free, but nothing else needs it
            # until the output DMAs.)
            with tc.tile_pool(name="mlp_sb", bufs=1) as mp, tc.tile_pool(
                name="ps6", bufs=1, space="PSUM"
            ) as ps6:
                xn2T8 = mp.tile([P, M1A, tq], F8)
                xn2T = mp.tile([P, N_H - M1A, tq], BF)
                w1a_sb = mp.tile([P, M1A, MLPD], F8)
                w1b_sb = mp.tile([P, N_H - M1A, MLPD], BF)
                w2_sb = mp.tile([P, N_M, H], F8)
                h1c = mp.tile([P, N_M, CH], F8)
                y2T = mp.tile([P, N_H, CH], BF)
                for i in range(M1A):
                    nc.sync.dma_start(out=w1a_sb[:, i, :], in_=w1a_d[i * P : (i + 1) * P, :])
                for i in range(N_H - M1A):
                    nc.sync.dma_start(out=w1b_sb[:, i, :], in_=w1b_d[i * P : (i + 1) * P, :])
                for i in range(N_M):
                    nc.sync.dma_start(out=w2_sb[:, i, :], in_=w2_d[i * P : (i + 1) * P, :])

                def ln2_tile(t):
                    xn2_bf = tmps.tile([P, H], BF, tag="xn_bf", bufs=6)
                    ln_tile(x1_sb[:, t, :], ln2w_bc, ln2b_bc, xn2_bf)
                    # paired evacuation (see xnT): pairs (0,1),(2,3) land in
                    # the fp8 tile, (4,5) in the bf16 tile
                    for j2 in range(0, N_H, 2):
                        pt = psum.tile([P, 2, P], BF, tag="aux", bufs=2)
                        nc.tensor.transpose(
                            pt[:, 0, :], xn2_bf[:, j2 * P : (j2 + 1) * P], ident
                        )
                        nc.tensor.transpose(
                            pt[:, 1, :], xn2_bf[:, (j2 + 1) * P : (j2 + 2) * P], ident
                        )
                        dst = (
                            xn2T8[:, j2 : j2 + 2, t * P : (t + 1) * P]
                            if j2 < M1A
                            else xn2T[:, 0:2, t * P : (t + 1) * P]
                        )
                        nc.vector.tensor_copy(out=dst, in_=pt)

                uT = mp.tile([P, N_H, tq], BF)

                def o_chunk(c):
                    for hot in range(N_H):
                        pu = psum.tile([P, CH], F32, tag="aux", bufs=2)
                        for hit in range(0, N_H, 2):
                            nc.tensor.matmul(
                                pu,
                                wo_sb[:, hit : hit + 2, hot * P : (hot + 1) * P],
                                ctxT[:, hit : hit + 2, c * CH : (c + 1) * CH],
                                start=(hit == 0), stop=(hit == N_H - 2),
                                perf_mode=DR,
                            )
                        nc.vector.tensor_scalar(
                            uT[:, hot, c * CH : (c + 1) * CH], pu,
                            1.0 / (WS * CS), bo_sb[:, hot : hot + 1],
                            AluOpType.mult, AluOpType.add,
                        )

                def res_ln2(c):
                    for t in range(c * n_b, (c + 1) * n_b):
                        for j2 in range(0, N_H, 2):
                            pt = psum.tile([P, 2, P], BF, tag="aux", bufs=2)
                            nc.tensor.transpose(
                                pt[:, 0, :], uT[:, j2, t * P : (t + 1) * P], ident,
                            )
                            nc.tensor.transpose(
                                pt[:, 1, :], uT[:, j2 + 1, t * P : (t + 1) * P], ident,
                            )
                            nc.vector.tensor_add(
                                x1_sb[:, t, j2 * P : (j2 + 2) * P],
                                pt,
                                xr_all[:, t, j2 * P : (j2 + 2) * P],
                            )
                    for t in range(c * n_b, (c + 1) * n_b):
                        ln2_tile(t)

                def mlp_ph(c):
                    for mt in range(N_M):
                        ph = ps6.tile([P, CH], F32, tag="pmm", bufs=4)
                        for hit in range(0, M1A, 2):
                            nc.tensor.matmul(
                                ph,
                                w1a_sb[:, hit : hit + 2, mt * P : (mt + 1) * P],
                                xn2T8[:, hit : hit + 2, c * CH : (c + 1) * CH],
                                start=(hit == 0), stop=False,
                                perf_mode=DR,
                            )
                        for hit in range(N_H - M1A):
                            nc.tensor.matmul(
                                ph,
                                w1b_sb[:, hit, mt * P : (mt + 1) * P],
                                xn2T[:, hit, c * CH : (c + 1) * CH],
                                start=False, stop=(hit == N_H - M1A - 1),
                            )
                        nc.scalar.activation(
                            out=h1c[:, mt, :], in_=ph,
                            func=getattr(AF, mlp_act), bias=b1_sb[:, mt : mt + 1],
                            scale=1.0 / WS,
                        )

                def mlp_tail(c):
                    for hot in range(N_H):
                        py = ps6.tile([P, CH], F32, tag="pmm", bufs=4)
                        for mt in range(0, N_M, 2):
                            nc.tensor.matmul(
                                py,
                                w2_sb[:, mt : mt + 2, hot * P : (hot + 1) * P],
                                h1c[:, mt : mt + 2, :],
                                start=(mt == 0), stop=(mt == N_M - 2),
                                perf_mode=DR,
                            )
                        nc.vector.tensor_scalar(
                            y2T[:, hot, :], py, 1.0 / WS, b2_sb[:, hot : hot + 1],
                            AluOpType.mult, AluOpType.add,
                        )
                    for b4 in range(n_b):
                        t = c * n_b + b4
                        outt = toks.tile([P, H], F32, tag="xtok")
                        for j2 in range(0, N_H, 2):
                            pt = psum.tile([P, 2, P], BF, tag="aux", bufs=2)
                            nc.tensor.transpose(
                                pt[:, 0, :], y2T[:, j2, b4 * P : (b4 + 1) * P], ident,
                            )
                            nc.tensor.transpose(
                                pt[:, 1, :], y2T[:, j2 + 1, b4 * P : (b4 + 1) * P], ident,
                            )
                            nc.vector.tensor_add(
                                outt[:, j2 * P : (j2 + 2) * P],
                                pt,
                                x1_sb[:, t, j2 * P : (j2 + 2) * P],
                            )
                        # alternate DMA queues so the output drain overlaps
                        eng = nc.sync if t % 2 == 0 else nc.gpsimd
                        eng.dma_start(out=out_d[t * P : (t + 1) * P, :], in_=outt)

                # interleaved emission: both out-proj chunks back-to-back
                # (36 DR matmuls keep PE dense while DVE evacuates), then
                # MLP chunk 0's first-linear groups fill the PE dip while
                # chunk 1's residual/LN2 chain resolves.
                o_chunk(0)
                o_chunk(1)
                res_ln2(0)
                mlp_ph(0)
                res_ln2(1)
                mlp_tail(0)
                mlp_ph(1)
                mlp_tail(1)

    nc.compile()
    _BUILD_CACHE[key] = nc
    return nc


def make_in_maps(inputs, tkv=S):
    """Build the 8 per-core input maps from full inputs."""
    f = np.asarray
    x = f(inputs["x"], dtype=np.float32)
    tq = tkv // 2
    wcast = {
        n: np.ascontiguousarray((f(inputs[n], dtype=np.float32) * WS).astype(NPF8))
        for n in ["wq", "wk", "wv", "wo", "w2"]
    }
    w1f = f(inputs["w1"], dtype=np.float32) * WS
    wcast["w1a"] = np.ascontiguousarray(w1f[: 4 * P].astype(NPF8))
    wcast["w1b"] = np.ascontiguousarray(w1f[4 * P :].astype(ml_dtypes.bfloat16))
    fp32v = {
        n: np.ascontiguousarray(f(inputs[n], dtype=np.float32))
        for n in ["bq", "bk", "bo", "b1", "b2"]
    }
    for n in ["ln1_w", "ln1_b", "ln2_w", "ln2_b"]:
        fp32v[n] = np.ascontiguousarray(f(inputs[n]).astype(ml_dtypes.bfloat16))
    # bv rides a ones-matmul row inside the x256-scaled V psum
    fp32v["bv"] = np.ascontiguousarray(
        (f(inputs["bv"], dtype=np.float32) * WS).astype(ml_dtypes.bfloat16)
    )
    in_maps = []
    for c in range(8):
        b, half = c // 2, c % 2
        if half == 0:
            x_loc = x[b, :tkv]
        else:
            x_loc = np.concatenate([x[b, tq:tkv], x[b, :tq]], axis=0)
        m = {"x_loc": np.ascontiguousarray(x_loc.astype(ml_dtypes.bfloat16))}
        m.update(wcast)
        m.update(fp32v)
        in_maps.append(m)
    return in_maps


def kernel(**inputs):
    f = np.asarray
    ln_triv = all(
        bool(np.all(f(inputs[w], dtype=np.float32) == 1.0)) for w in ("ln1_w", "ln2_w")
    ) and all(
        bool(np.all(f(inputs[b], dtype=np.float32) == 0.0)) for b in ("ln1_b", "ln2_b")
    )
    bias_triv = bool(np.all(f(inputs["bv"], dtype=np.float32) == 0.0))
    nc = build(S, ln_trivial=ln_triv, bias_trivial=bias_triv)
    in_maps = make_in_maps(inputs, S)
    res = run_bass_kernel_spmd(nc, in_maps, core_ids=list(range(8)))
    tq = S // 2
    out = np.empty((B, S, H), dtype=np.float32)
    for c in range(8):
        b, half = c // 2, c % 2
        out[b, half * tq : (half + 1) * tq] = res.results[c]["out_loc"]
    return out
